# revision 9
# baseline (speedup 1.0000x reference)
"""Trainium2 Bass kernel for nn_BitKHopSampler.

Computes out[b, s, v] = y[b, v] + (1 - 2*y[b, v]) * mag[b, s, v] where
mag[b, s, v] = 1 iff v appears in idx[b, s, :].  Equivalently: broadcast
y[b, :] over samples, then flip each hit position v -> 1 - y[b, v].

Sharding: pure data parallel over the batch dim, 64 batches per core on
8 cores.  The kernel is DMA-write-bound (32 MiB fp32 output per core);
the structure below is chosen so every output dma_start moves 16 KiB of
DRAM-contiguous data per partition (4 consecutive sample rows), which
keeps all 16 SDMA engines in lockstep at ~413 GB/s.  (With 4 KiB
packets, SDMA engine 15 runs ~15-20% slower than the rest and finishes
the kernel alone, adding ~10 us.)

Per core (all shapes hardcoded):
  - yspl  (128, 2048) bf16: partition 32w+4r+j, col block c holds bf16
          split j (hi/mid/lo/ones) of y[32c+8w+r]; y is re-encoded
          losslessly as three bf16 summands.
  - lall  (128, 768)  bf16 matmul selectors (see _make_lallx)
  - idx16 (128, 260)  int16 scatter indices, duplicate hops -> -1
  - out   (64*128, 1024) fp32

Device pipeline: batches 0-3 run solo ([128 samples, V] tiles, batch 0
split into 512-col halves) so output DMA starts as early as possible;
batches 4..63 run in groups of 4 where partition p = 32c+q holds
samples 4q..4q+3 of batch 4g+c:
  PE    : ybc = broadcast of y[batch(p)] to partitions via K=32 bf16
          matmul with a per-partition batch-selector lhsT
  GPSIMD: local_scatter builds the int16 hit mask from idx16
  DVE   : py -= mask  (PSUM in place)
  ACT   : out_tile = |py|  (PSUM -> SBUF; equals y or 1-y exactly)
  DMA   : 2 MiB per group, 16 KiB contiguous per partition
"""

import numpy as np

import concourse.bacc as bacc
import concourse.bass as bass
import concourse.tile as tile
from concourse import mybir
from concourse.bass_utils import run_bass_kernel_spmd


B, S, V, H = 512, 128, 1024, 4
NCORES = 8
BL = B // NCORES  # 64 batches per core
NG = BL // 4  # 16 groups of 4 batches; group 0 handled as solos

_nc_cache = None


def _build_bass():
    nc = bacc.Bacc("TRN2", debug=False, enable_asserts=False, num_devices=NCORES)
    ysl_d = nc.dram_tensor(
        "ysl", [96, 4096], mybir.dt.bfloat16, kind="ExternalInput"
    ).ap()
    idx_d = nc.dram_tensor(
        "idx16", [S, 276], mybir.dt.int16, kind="ExternalInput"
    ).ap()
    out_d = nc.dram_tensor(
        "out", [BL * S, V], mybir.dt.float32, kind="ExternalOutput"
    ).ap()

    f32 = mybir.dt.float32
    bf16 = mybir.dt.bfloat16
    Op = mybir.AluOpType

    with tile.TileContext(nc) as tc:
        with (
            tc.tile_pool(name="const", bufs=1) as cp,
            tc.tile_pool(name="outp", bufs=3) as outp,
            tc.tile_pool(name="outh", bufs=3) as outh,
            tc.tile_pool(name="maskp", bufs=8) as maskp,
            tc.tile_pool(name="ps", bufs=4, space="PSUM") as psp,
        ):
            # ---- warmups (no data deps, run immediately) ----
            # Dummy scatter: forces Bacc's ModifyPoolConfig + the ~2.5us
            # gpsimd library IRAM load to the front, overlapping input DMAs.
            DUMIDX = cp.tile([S, 2], mybir.dt.int16, tag="DUMIDX")
            nc.gpsimd.memset(DUMIDX[:], -1)
            DUMSC = cp.tile([S, 2], mybir.dt.int16, tag="DUMSC")
            nc.gpsimd.local_scatter(
                out_ap=DUMSC[:],
                data_ap=DUMIDX[:],
                idxs_ap=DUMIDX[:],
                channels=S,
                num_elems=2,
                num_idxs=2,
            )

            # ---- input loads ----
            # sync ring carries the starter slices (everything batch 0-7
            # needs), scalar ring the remainders; both issue immediately
            # so the first matmul can start ~2 us after the preamble.
            # One merged tensor, all on the sync ring: HWDGE rings drain
            # FIFO per engine, so the starter slice (everything batches 0-7
            # need: ys col-block 0 + all selectors, 4 KiB/partition) lands
            # first; idx next; the rest streams in behind while batches 0-7
            # compute.  Per-partition runs are 4-8 KiB (fat packets).
            IDX = cp.tile([S, 276], mybir.dt.int16, tag="IDX")
            YSL = cp.tile([128, 4096], bf16, tag="YSL")
            # starter on sync, idx on scalar: the two rings drain in
            # parallel, so the starter (gates the first matmul) is not
            # serialized behind the idx load.
            nc.scalar.dma_start(out=IDX[:], in_=idx_d[:])
            nc.sync.dma_start(out=YSL[0:32, 0:2048], in_=ysl_d[0:32, 0:2048])
            nc.sync.dma_start(out=YSL[0:32, 2048:4096], in_=ysl_d[0:32, 2048:4096])
            nc.sync.dma_start(out=YSL[32:96, :], in_=ysl_d[32:96, :])

            # Dummy Abs AFTER the dma issues: the auto-inserted ACT table
            # load (~1.3us) must not delay the scalar-ring idx issue.
            DUMF = cp.tile([S, 2], f32, tag="DUMF")
            nc.vector.memset(DUMF[:], 0.0)
            DUMF2 = cp.tile([S, 2], f32, tag="DUMF2")
            nc.scalar.activation(
                out=DUMF2[:], in_=DUMF[:], func=mybir.ActivationFunctionType.Abs
            )

            # Scatter payload + wait-absorbers: InstISA (local_scatter) only
            # supports a limited number of semaphore waits, so satisfy its
            # cross-engine deps (IDX DMA, ONES init) on the gpsimd engine
            # itself; program order then covers them for every scatter.
            ONES = cp.tile([S, H], mybir.dt.int16, tag="ONES")  # scatter payload
            nc.gpsimd.memset(ONES[:], 1)
            IDXPROBE = cp.tile([S, 2], mybir.dt.int16, tag="IDXPROBE")
            nc.gpsimd.tensor_copy(out=IDXPROBE[:], in_=IDX[:, 0:2])

            # out[s, v] = |ybc[s, v] - mask[s, v]|.  With mask in {0, 1} and
            # y in [0, 1) this equals y (no hit) or 1-y (hit).
            def broadcast(py_slice, lhsT, rhs_cols, width):
                for h2 in range(width // 512):
                    nc.tensor.matmul(
                        out=py_slice[:, h2 * 512 : (h2 + 1) * 512],
                        lhsT=lhsT,
                        rhs=YSL[
                            rhs_cols[0] : rhs_cols[0] + 32,
                            rhs_cols[1] + h2 * 512 : rhs_cols[1] + (h2 + 1) * 512,
                        ],
                        start=True,
                        stop=True,
                    )

            def block(py_slice, idx_col, ot_slice, width):
                # one [128, width] unit: scatter -> sub -> abs
                mk = maskp.tile([S, V], mybir.dt.int16)
                nc.gpsimd.local_scatter(
                    out_ap=mk[:, 0:width],
                    data_ap=ONES[:],
                    idxs_ap=IDX[:, idx_col : idx_col + H],
                    channels=S,
                    num_elems=width,
                    num_idxs=H,
                )
                nc.vector.tensor_tensor(
                    out=ot_slice, in0=py_slice[:], in1=mk[:, 0:width], op=Op.subtract
                )
                nc.scalar.activation(
                    out=ot_slice,
                    in_=ot_slice,
                    func=mybir.ActivationFunctionType.Abs,
                )

            def block2(py, idx_col0, idx_col1, ot):
                # fused pair of [128, V] units sharing one broadcast: two
                # scatters (local_scatter caps num_elems at 1024), then ONE
                # 2V-wide DVE subtract (py read twice via 0-stride) and ONE
                # 2V-wide ACT abs — halves per-instruction overhead on the
                # two full-tile engines that pace the pipeline.
                mk = maskp.tile([S, 2 * V], mybir.dt.int16, tag="mk2")
                for u in range(2):
                    nc.gpsimd.local_scatter(
                        out_ap=mk[:, u * V : (u + 1) * V],
                        data_ap=ONES[:],
                        idxs_ap=IDX[:, (idx_col0, idx_col1)[u] : (idx_col0, idx_col1)[u] + H],
                        channels=S,
                        num_elems=V,
                        num_idxs=H,
                    )
                py_b = py[:].rearrange("s (u v) -> s u v", u=1).broadcast_to([S, 2, V])
                nc.vector.tensor_tensor(
                    out=ot[:].rearrange("s (u v) -> s u v", u=2),
                    in0=py_b,
                    in1=mk[:].rearrange("s (u v) -> s u v", u=2),
                    op=Op.subtract,
                )
                nc.scalar.activation(
                    out=ot[:],
                    in_=ot[:],
                    func=mybir.ActivationFunctionType.Abs,
                )

            # ---- solo batches 0-3 (window w=0, col block c=0) ----
            # batch 0 in two 512-col halves to get the first output DMA
            # issued as early as possible; the DMA engines are idle until
            # the steady stream arrives, so small early DMAs are free.
            ot_s = outp.tile([S, 4 * V], f32, name="ot", tag="ot")
            for j in range(2):
                py = psp.tile([S, V], f32)
                broadcast(py[:, 0:512], YSL[0:32, 1024:1152], (0, j * 512), 512)
                block(
                    py[:, 0:512],
                    4 * j,
                    ot_s[:, j * 512 : (j + 1) * 512],
                    512,
                )
                nc.sync.dma_start(
                    out=out_d[0:S, j * 512 : (j + 1) * 512],
                    in_=ot_s[:, j * 512 : (j + 1) * 512],
                )
            for b in range(1, 4):
                py = psp.tile([S, V], f32)
                broadcast(py[:], YSL[0:32, 1024 + b * 128 : 1024 + (b + 1) * 128], (0, 0), V)
                block(
                    py[:],
                    8 + 4 * (b - 1),
                    ot_s[:, b * V : (b + 1) * V],
                    V,
                )
                nc.sync.dma_start(
                    out=out_d[b * S : (b + 1) * S, :],
                    in_=ot_s[:, b * V : (b + 1) * V],
                )

            # ---- pairs (4,5) and (6,7): partition p = 64c+q holds samples
            # 2q, 2q+1 of batch b0+c; 8 KiB contiguous DRAM per partition.
            # Bridges the solo->group granularity seam so the DMA stream
            # never starves while the first 4-batch group accumulates.
            # The PSUM broadcast is identical across a pair's two sample
            # tiles, so it is computed once and read twice. ----
            for pi in range(2):
                b0 = 4 + 2 * pi
                otp2 = outh.tile([S, 2 * V], f32, name="oth", tag="oth")
                py = psp.tile([S, V], f32)
                broadcast(py[:], YSL[0:32, 1792 + pi * 128 : 1792 + (pi + 1) * 128], (0, 0), V)
                block2(py, 260 + 8 * pi, 260 + 8 * pi + 4, otp2)
                nc.sync.dma_start(
                    out=out_d[b0 * S : (b0 + 2) * S, :].rearrange(
                        "(c q t) v -> (c q) (t v)", c=2, q=64, t=2
                    ),
                    in_=otp2[:],
                )

            # ---- groups of 4 batches: partition p = 32c+q holds samples
            # 4q..4q+3 of batch 4g+c.  One shared PSUM broadcast per group
            # (read 4x).  Output in two column-half DMAs so every packet is
            # 8 KiB of contiguous DRAM per partition: the SDMA engines
            # sustain ~26.5 GB/s each on 8 KiB packets vs ~22 GB/s on
            # 16 KiB, lifting the aggregate write roofline ~18%.  The 8 KiB
            # fast mode only holds while the queue backlog stays under
            # ~2.5 MiB (deeper backlog -> ~375 ns/packet), so output tiles
            # live in a small half-group pool (bufs=3): reusing a tile
            # waits on its previous DMA, pacing descriptor issue to at
            # most ~3 MiB ahead of the engines. ----
            for g in range(2, NG):
                b0 = 4 * g
                cblk = b0 // 24
                w = (b0 % 24) // 8
                base = 32 * w
                rhs_base = (0, 2048, 3072)[cblk]
                par = (b0 % 8) // 4  # 0: slots 0-3, 1: slots 4-7
                lhsT = YSL[base : base + 32, 1536 + par * 128 : 1536 + (par + 1) * 128]
                py = psp.tile([S, V], f32)
                broadcast(py[:], lhsT, (base, rhs_base), V)
                for hf in range(2):
                    ot = outh.tile([S, 2 * V], f32, name="oth", tag="oth")
                    ic = 20 + 16 * (g - 1) + 8 * hf
                    block2(py, ic, ic + 4, ot)
                    nc.sync.dma_start(
                        out=out_d[b0 * S : (b0 + 4) * S, :].rearrange(
                            "(c q u t) v -> (c q) u (t v)", c=4, q=32, u=2, t=2
                        )[:, hf, :],
                        in_=ot[:],
                    )
    nc.compile()
    return nc


def _get_nc():
    global _nc_cache
    if _nc_cache is None:
        _nc_cache = _build_bass()
    return _nc_cache


def _make_lallx():
    import ml_dtypes

    la = np.zeros((128, 1024), np.float32)
    for wb in (0, 32, 64):
        # solo selectors: block b (cols 128b..), all cols pick slot b
        for b in range(4):
            la[wb + 4 * b : wb + 4 * b + 3, 128 * b : 128 * (b + 1)] = 1.0
        # group selectors: col 512 + par*128 + p picks slot 4*par + p//32
        for par in range(2):
            for c in range(4):
                r = 4 * par + c
                la[
                    wb + 4 * r : wb + 4 * r + 3,
                    512 + par * 128 + 32 * c : 512 + par * 128 + 32 * (c + 1),
                ] = 1.0
        # pair selectors: col 768 + pi*128 + p picks slot 4 + 2*pi + p//64
        for pi in range(2):
            for c in range(2):
                r = 4 + 2 * pi + c
                la[
                    wb + 4 * r : wb + 4 * r + 3,
                    768 + pi * 128 + 64 * c : 768 + pi * 128 + 64 * (c + 1),
                ] = 1.0
    return np.ascontiguousarray(la.astype(ml_dtypes.bfloat16))


def _prep_inputs(y, idx):
    """Slice the full inputs into per-core in_maps (host-side index massaging
    only: dtype narrowing, layout permutation, duplicate-hop sentinel)."""
    import ml_dtypes

    y = np.asarray(y, dtype=np.float32)
    ii = np.asarray(idx)
    i16 = ii.astype(np.int16)  # values in [0, 1024)
    # reference uses .set semantics: mark duplicate hops within a row so the
    # scatter writes each position once; local_scatter ignores negatives.
    dup = np.zeros(ii.shape, dtype=bool)
    for j in range(1, H):
        for k in range(j):
            dup[..., j] |= ii[..., j] == ii[..., k]
    i16[dup] = -1

    bf = ml_dtypes.bfloat16
    hi = y.astype(bf)
    r1 = y - hi.astype(np.float32)
    mid = r1.astype(bf)
    lo = (r1 - mid.astype(np.float32)).astype(bf)  # exact: <=8 bits remain
    ones = np.ones_like(hi)
    yspl = np.stack([hi, mid, lo, ones], axis=1)  # (B, 4, V) bf16

    lall = _make_lallx()

    in_maps = []
    for core in range(NCORES):
        yb = yspl[core * BL : (core + 1) * BL]  # (64, 4, V)
        ib = i16[core * BL : (core + 1) * BL]  # (64, S, H)
        # ysl layout: partition 32w+4r+j (w<3); per-partition cols:
        # [ys c0 | selectors | ys c1 | ys c2], c = split j of y[24c+8w+r]
        ys = np.zeros((96, 4096), bf)
        ys[:, 1024:2048] = lall[0:96]
        for c, col in ((0, 0), (1, 2048), (2, 3072)):
            blk = yb[24 * c : min(24 * (c + 1), BL)]  # (<=24, 4, V)
            n = blk.shape[0] * 4
            ys[0:n, col : col + V] = blk.reshape(n, V)
        # idx layout (128, 260) int16
        ix = np.full((S, 276), -1, np.int16)
        # batch 0 halves
        v0 = ib[0]  # (S, H)
        loh = np.where((v0 >= 0) & (v0 < 512), v0, -1)
        hih = np.where(v0 >= 512, v0 - 512, -1)
        ix[:, 0:4] = loh
        ix[:, 4:8] = hih
        for b in range(1, 4):
            ix[:, 8 + 4 * (b - 1) : 8 + 4 * b] = ib[b]
        # groups: col 20+16(g-1)+4t+h, channel p: idx[4g+p//32, 4*(p%32)+t, h]
        for g in range(1, NG):
            blk = ib[4 * g : 4 * g + 4]  # (4, S, H)
            # want arr[p, t, h] = blk[p//32, 4*(p%32)+t, h]
            a = blk.reshape(4, 32, 4, H)  # (c, q, t, h)
            ix[:, 20 + 16 * (g - 1) : 20 + 16 * g] = a.reshape(128, 16)
        # pairs (4,5), (6,7): col 260+8pi+4t+h, channel p: idx[b0+p//64, 2*(p%64)+t, h]
        for pi in range(2):
            b0 = 4 + 2 * pi
            a = ib[b0 : b0 + 2].reshape(2, 64, 2, H)  # (c, q, t, h)
            ix[:, 260 + 8 * pi : 260 + 8 * (pi + 1)] = a.reshape(128, 8)
        in_maps.append(
            {
                "ysl": np.ascontiguousarray(ys),
                "idx16": np.ascontiguousarray(ix),
            }
        )
    return in_maps


def _run(y, idx, **spmd_kwargs):
    nc = _get_nc()
    in_maps = _prep_inputs(y, idx)
    res = run_bass_kernel_spmd(nc, in_maps, core_ids=list(range(NCORES)), **spmd_kwargs)
    out = np.empty((B, S, V), dtype=np.float32)
    for c in range(NCORES):
        out[c * BL : (c + 1) * BL] = res.results[c]["out"].reshape(BL, S, V)
    return out, res


def kernel(a=None, b=None, c=None, y=None, idx=None, **_unused):
    # a, b, c are unused by the reference computation.
    out, _ = _run(y, idx)
    return out

